# revision 54
# baseline (speedup 1.0000x reference)
"""EdgeConv block (KNN + gather + 2-layer edge MLP + max-pool) on 8 Trainium2 cores.

Data-parallel over batch: core c processes one point cloud ([4096, 64]).

Per-core pipeline (all on device), v3:
  - Ranking proxy 2*x_i.x_j - |x_j|^2 (the -|x_i|^2 term is constant per row
    so it cannot change the per-row top-k; dropping it removes every
    partition-65 aug row and all [1, N] staging DMAs).  65-dim f32r PE
    matmuls (1 cyc/col vs 4 for f32; |err| ~1.4e-4 rel).  The f32r tables
    are written straight from the transpose PSUM (ACT copy rounds; no f32
    staging or separate rounding pass); x is loaded with ONE batched DMA.
  - Top-16 per row: 8 chunks of 512; DVE max8 + max_index per chunk give
    top-8 candidates (end-to-end rel err of chunked candidates: 1.9e-3).
    Without -|x_i|^2 the diagonal is provably each row's max, i.e. exactly
    the top-1 candidate of its own chunk: one [128, 1] memset kills it.
    Level 2: max8/match_replace/max8 -> tau; rp = (vals >= tau) * (N - j)
    ranked by max8 twice -> exact top-16 with lowest-j tie-break.
  - Gather via TWO InstDMAGatherAnt (1024 idx each); v table shipped as
    duplicated bf16 rows [v_j; v_j] so each element stays 256B (gather
    minimum) while vg lands in bf16.  The int16 idx wrap table is built by
    a shuffled 4KB DRAM round-trip, a PE stripe-broadcast matmul, and one
    ACT shuffle-convert copy.
  - Edge MLP layer-1 factorized: pre1(i,k) = vg + u_i broadcast as a bf16
    DVE TT (2x perf mode), GELU on ACT -> h1 bf16.  h1 PE-transposed (4
    blocks per PSUM bank, one ACT copy each).  Layer-2 as 4 bf16 matmuls
    with k-parity on partition halves; ONE [128, 1024] GELU (ACT cost is
    free-size cycles, so 64-partition ops waste half the engine).  Max
    over K: transpose h2 FIRST (8 bf16 PE transposes into one PSUM bank),
    then a pure free-dim bf16 TT fold tree [128,1024] -> [128,64] at 2x,
    which also removes the old transpose-back step.  Last fold emits f32
    rows, HWDGE out.

  The i-tile loop is software-pipelined 2 deep (stage1: distance/topk/idx/
  gather; stage2: MLP) so the ~12us idx-shuffle+gather latency of tile t
  hides behind the MLP work of tiles t-2/t-1.  In repeat (profiling) mode
  the big tables are rep-parity double-buffered so rep r+1's prelude
  overlaps rep r's drain.
"""

import sys

if "/opt/trn_rl_repo" not in sys.path:
    sys.path.insert(0, "/opt/trn_rl_repo")

import ml_dtypes
import numpy as np

import bass_rust
import concourse.bass as bass
import concourse.mybir as mybir
from concourse import library_config
from concourse.bass_utils import run_bass_kernel_spmd
from concourse.tile import TileContext
from concourse.vector_clock import ScopedClock

B, N, C, D, K = 8, 4096, 64, 64, 16
# augmented contraction dim: [2x; 1] vs [x_j; -sq_j].  The -sq_i term of true
# negd2 is constant per row i, so it cannot change the per-row top-k -- drop
# it (also avoids any partition-65 aug row, which no engine AP can address).
CAUG = C + 1
NT = N // 128         # 32 i-tiles of 128 points
CH = 512              # candidate chunk length
NCH = N // CH         # 8 chunks per row
NCAND = 8 * NCH       # 64 level-1 candidates
F32 = mybir.dt.float32
F32R = mybir.dt.float32r
BF16 = mybir.dt.bfloat16
I16 = mybir.dt.int16
U16 = mybir.dt.uint16
AF = mybir.ActivationFunctionType
ALU = mybir.AluOpType

import os
DIST_F32R = int(os.environ.get("KM_DIST_F32R", "1"))   # 1: f32r (4x PE), 0: f32


class _TC(TileContext):
    """TileContext whose exit drain splits its sem waits across single-wait
    NOPs: this walrus build rejects >~2 sync waits on one SP instruction."""

    def _drain_and_barrier(self, tick_clock, wait_clock):
        gc = list(tick_clock.global_clock)
        for p, v in enumerate(gc):
            if v > 0:
                sub = [0] * len(gc)
                sub[p] = v
                nop = self.nc.sync.nop()
                wait_clock.add_sem_waits(
                    nop.ins, ScopedClock({None: bass_rust.VectorClock(sub)})
                )
        self.nc.sync.drain()
        self.nc.all_engine_barrier()
        popped = self.nc._tile_sem_poison_stack.pop()
        assert popped is self._sem_poison
        self.nc.clear_and_free_semaphores(list(self.sems.allocated().values()))
        self.nc.all_engine_barrier()


def host_constants(W1, b1, W2, b2):
    """Host-side constant tensors shipped to every core."""
    W1 = np.asarray(W1, np.float32)
    W2 = np.asarray(W2, np.float32)
    b2 = np.asarray(b2, np.float32)
    # uW applied against lhs_aug = [2x; 1]: rows 0..C-1 scaled 0.5 to undo
    # the 2x, row C carries b1 (so u = x@(W1a-W1b) + b1).
    uW = np.zeros((CAUG, D), np.float32)
    uW[:C] = 0.5 * (W1[:C] - W1[C:])
    uW[C] = np.asarray(b1, np.float32)
    # revb[p, f] = N - CH*(f//8): base for rev-index payloads per candidate
    revb = (N - CH * (np.arange(NCAND) // 8))[None, :] * np.ones((128, 1))
    # s16[ch, p] = 1 iff p % 16 == ch (idx-table stripe broadcast)
    s16 = (np.arange(128)[None, :] % 16 == np.arange(16)[:, None])
    consts = {
        "uW": uW,
        "vW": np.ascontiguousarray(W1[C:]),                     # [C, D]
        "W2db": np.concatenate([W2, W2], 0).astype(ml_dtypes.bfloat16),
        "b2d": np.concatenate([b2, b2]).reshape(128, 1).astype(np.float32),
        "idf": np.eye(128, dtype=np.float32),
        "idb": np.eye(128, dtype=np.float32).astype(ml_dtypes.bfloat16),
        "dgm": (1e30 * np.eye(128, dtype=np.float32)),
        "revb": revb.astype(np.float32),
        "s16": s16.astype(np.float32),
        "nonesc": -np.ones((C, 1), np.float32),
    }
    return consts


def _split_excess_waits(nc, max_waits=1):
    """Hoist excess sync waits onto same-engine NOPs (this walrus build
    rejects instructions carrying more than one sync wait)."""
    ctr = 0
    for f in nc.m.functions:
        for bb in f.blocks:
            out = []
            for ins in bb.instructions:
                si = ins.sync_info
                waits = list(si.on_wait) if si is not None and si.on_wait else []
                if len(waits) > max_waits:
                    excess, keep = waits[:-max_waits], waits[-max_waits:]
                    for i in range(0, len(excess), max_waits):
                        chunk = excess[i:i + max_waits]
                        nop = mybir.InstNoOp(
                            name=f"WS-{ctr}", engine=ins.engine, ins=[], outs=[],
                            sync_info=mybir.SyncInfo(on_wait=chunk, on_update=[]),
                        )
                        nc.register_instruction(nop, overwrite=True)
                        out.append(nop)
                        ctr += 1
                    ins.sync_info = mybir.SyncInfo(
                        on_wait=keep,
                        on_update=list(si.on_update) if si.on_update else [],
                    )
                out.append(ins)
            bb.instructions[:] = out


def build_nc(repeat=1):
    nc = bass.Bass("TRN2", target_bir_lowering=False, debug=False, num_devices=B,
                   num_swdge_queues=4, dynamic_dma_scratch_size=65536)
    x = nc.dram_tensor("x", [N, C], F32, kind="ExternalInput").ap()
    y = nc.dram_tensor("y", [N, D], F32, kind="ExternalOutput").ap()
    cin = {
        name: nc.dram_tensor(name, list(shape), dt, kind="ExternalInput").ap()
        for name, dt, shape in [
            ("uW", F32, (CAUG, D)), ("vW", F32, (C, D)),
            ("W2db", BF16, (128, D)), ("b2d", F32, (128, 1)),
            ("idf", F32, (128, 128)), ("idb", BF16, (128, 128)),
            ("dgm", F32, (128, 128)), ("revb", F32, (128, NCAND)),
            ("s16", F32, (16, 128)), ("nonesc", F32, (C, 1)),
        ]
    }

    with _TC(nc) as tc, \
         tc.tile_pool(name="const", bufs=1) as cp, \
         tc.tile_pool(name="big", bufs=1) as big, \
         tc.tile_pool(name="dram", bufs=1, space="DRAM") as dramp:
        sb = {name: cp.tile_from(ap, name=f"c_{name}") for name, ap in cin.items()}
        nc.gpsimd.load_library(library_config.mlp)
        nidx_reg = nc.gpsimd.to_reg(1024)

        DIST = F32R if DIST_F32R else F32
        # rep-parity double buffers: rep r+1's table writes would otherwise
        # WAR-serialize against rep r's last matmul/gather reads
        NB = 2 if repeat > 1 else 1
        rhs_rb = [big.tile([CAUG, N], DIST, name=f"rhsr{i}")
                  for i in range(NB)]                            # [x_j; -sq_j]
        lhs_rb = [big.tile([CAUG, N], DIST, name=f"lhsr{i}")
                  for i in range(NB)]                            # [2x_i; 1]
        u_rb = [big.tile([128, NT * D], BF16, name=f"ur{i}") for i in range(NB)]
        # v table rows duplicated [v_j; v_j] bf16 so each gather element stays
        # 256B (gather constraint) while vg lands in bf16 -> pre1 TT runs in
        # the 2x DVE perf mode (all operands 2-byte)
        v_dramb = [dramp.tile([N, 2 * C], BF16, name=f"vdr{i}")
                   for i in range(NB)]

        for rep in range(repeat):
            rhs_r, lhs_r = rhs_rb[rep % NB], lhs_rb[rep % NB]
            u_r, v_dram = u_rb[rep % NB], v_dramb[rep % NB]
            # ---------------- setup ----------------
            with tc.tile_pool(name=f"sst{rep}", bufs=1) as sst, \
                 tc.tile_pool(name=f"sup{rep}", bufs=4) as sup, \
                 tc.tile_pool(name=f"sps{rep}", bufs=2, space="PSUM") as sps, \
                 tc.tile_pool(name=f"spu{rep}", bufs=1, space="PSUM") as spu, \
                 tc.tile_pool(name=f"sxq{rep}", bufs=1) as sxq:
                # one batched x load (32 separate DMAs would serialize ~18us
                # of SP dispatch in the prelude)
                xall = sst.tile([128, NT * C], F32)
                nc.sync.dma_start(
                    out=xall.rearrange("p (t c) -> p t c", c=C),
                    in_=x.rearrange("(t p) c -> p t c", p=128))
                # transpose 4 i-tiles per PSUM group; write the f32r aug
                # tables DIRECTLY from PSUM (ACT copy rounds; no f32 staging
                # or separate rounding pass), xsq on DVE from the same PSUM
                xsq = sxq.tile([C, N], F32, tag="xs")
                for g in range(NT // 4):
                    tp4 = sps.tile([C, 512], F32, tag="tp4")
                    for q in range(4):
                        t4 = 4 * g + q
                        nc.tensor.transpose(
                            tp4[:, 128 * q:128 * (q + 1)],
                            xall[:, C * t4:C * (t4 + 1)], sb["idf"])
                    gs = slice(512 * g, 512 * (g + 1))
                    nc.scalar.activation(rhs_r[0:C, gs], tp4, AF.Copy)
                    nc.scalar.activation(lhs_r[0:C, gs], tp4, AF.Copy, scale=2.0)
                    # square: PSUM x times the rounded SBUF copy (only one TT
                    # operand may be PSUM); |err| ~1.4e-4 rel, irrelevant here
                    nc.vector.tensor_tensor(
                        out=xsq[:, gs], in0=tp4,
                        in1=rhs_r[0:C, gs].bitcast(F32), op=ALU.mult)
                # ones-row of lhs via ACT const-fill (scale=0, bias=1);
                # partition base C=64 is legal
                nc.scalar.activation(lhs_r[C:C + 1, :], xsq[0:1, :],
                                     AF.Copy, scale=0.0, bias=1.0)
                for h in range(2):
                    sqp = spu.tile([1, N // 2], F32, tag="uv")
                    for s in range(4):
                        c0 = 512 * s
                        nc.tensor.matmul(
                            sqp[:, c0:c0 + 512], lhsT=sb["nonesc"],
                            rhs=xsq[:, 2048 * h + c0:2048 * h + c0 + 512],
                            start=True, stop=True)
                    # sqp = -sq_j: straight into rhs row 64 (legal base)
                    nc.scalar.activation(
                        rhs_r[C:C + 1, 2048 * h:2048 * (h + 1)], sqp, AF.Copy)
                # u (from lhs_r so the ones-row carries b1) and v (staged in
                # SBUF then ONE batched DMA to the DRAM gather table: 64
                # per-tile DMAs would cost ~36us of SP).  Reading the rounded
                # f32r tables (bitcast) costs ~1.4e-4 rel on u/v: negligible.
                v_sb = sst.tile([128, NT * 2 * C], BF16)
                for t in range(NT):
                    i0 = 128 * t
                    upr = sps.tile([128, D], F32, tag="tp")
                    nc.tensor.matmul(
                        upr, lhsT=lhs_r[:, i0:i0 + 128].bitcast(F32),
                        rhs=sb["uW"], start=True, stop=True)
                    nc.vector.tensor_copy(u_r[:, D * t:D * (t + 1)], upr)
                    vpr = sps.tile([128, D], F32, tag="tp")
                    nc.tensor.matmul(
                        vpr, lhsT=rhs_r[0:C, i0:i0 + 128].bitcast(F32),
                        rhs=sb["vW"], start=True, stop=True)
                    vo = 2 * C * t
                    if t % 2 == 0:
                        nc.scalar.activation(v_sb[:, vo:vo + C], vpr, AF.Copy)
                        nc.vector.tensor_copy(v_sb[:, vo + C:vo + 2 * C], vpr)
                    else:
                        nc.vector.tensor_copy(v_sb[:, vo:vo + C], vpr)
                        nc.scalar.activation(v_sb[:, vo + C:vo + 2 * C], vpr,
                                             AF.Copy)
                nc.sync.dma_start(
                    out=v_dram.rearrange("(t p) dd -> p t dd", p=128),
                    in_=v_sb.rearrange("p (t dd) -> p t dd", dd=2 * C))

            # ---------------- main loop ----------------
            with tc.tile_pool(name=f"nd{rep}", bufs=2) as ndp, \
                 tc.tile_pool(name=f"sm{rep}", bufs=3) as smp, \
                 tc.tile_pool(name=f"ed{rep}", bufs=2) as edp, \
                 tc.tile_pool(name=f"vgp{rep}", bufs=3) as vgp, \
                 tc.tile_pool(name=f"ix{rep}", bufs=3) as ixp, \
                 tc.tile_pool(name=f"orp{rep}", bufs=3) as orp, \
                 tc.tile_pool(name=f"pq{rep}", bufs=2, space="PSUM") as pqp, \
                 tc.tile_pool(name=f"p2{rep}", bufs=1, space="PSUM") as p2p, \
                 tc.tile_pool(name=f"pt2{rep}", bufs=1, space="PSUM") as ptr2, \
                 tc.tile_pool(name=f"pib{rep}", bufs=1, space="PSUM") as pibp, \
                 tc.tile_pool(name=f"ptr{rep}", bufs=1, space="PSUM") as ptrp, \
                 tc.tile_pool(name=f"idd{rep}", bufs=3, space="DRAM") as iddp:
                pend = {}

                def stage1(t):
                    i0 = 128 * t
                    nd = ndp.tile([128, N], F32, tag="nd")
                    # distances (512-col chunks; 2 PSUM bufs = 2 banks, which
                    # leaves room for the h2-transpose bank)
                    for q in range(8):
                        pq = pqp.tile([128, 512], F32, tag="pq")
                        nc.tensor.matmul(
                            pq, lhsT=lhs_r[:, i0:i0 + 128],
                            rhs=rhs_r[:, 512 * q:512 * (q + 1)],
                            start=True, stop=True)
                        nc.scalar.activation(nd[:, 512 * q:512 * (q + 1)], pq,
                                             AF.Copy)
                    # level-1 top-8 per 512-chunk
                    vals = smp.tile([128, NCAND], F32, tag="vals")
                    gidx = smp.tile([128, NCAND], U16, tag="gidx")
                    for c in range(NCH):
                        nc.vector.max(vals[:, 8 * c:8 * c + 8],
                                      nd[:, CH * c:CH * (c + 1)])
                        nc.vector.max_index(
                            gidx[:, 8 * c:8 * c + 8], vals[:, 8 * c:8 * c + 8],
                            nd[:, CH * c:CH * (c + 1)])
                    # self-distance kill: without the -sq_i term the diagonal
                    # is provably each row's maximum (2x.y - sq_j <= sq_i), so
                    # it is exactly the top-1 candidate of its chunk -- zap
                    # that one slot instead of a [128,128] subtract.
                    nc.vector.memset(vals[:, 8 * (t // 4):8 * (t // 4) + 1],
                                     -3e38)
                    # level-2: exact top-16 with self-indexing payload
                    t8a = smp.tile([128, 8], F32, tag="t8a")
                    valsb = smp.tile([128, NCAND], F32, tag="scr")
                    t8b = smp.tile([128, 8], F32, tag="t8b")
                    nc.vector.max(t8a, vals)
                    nc.vector.match_replace(valsb, t8a, vals, -3e38)
                    nc.vector.max(t8b, valsb)
                    revi = smp.tile([128, NCAND], F32, tag="revi")
                    nc.vector.tensor_tensor(
                        out=revi, in0=sb["revb"], in1=gidx, op=ALU.subtract)
                    rp = smp.tile([128, NCAND], F32, tag="rp")
                    nc.vector.scalar_tensor_tensor(
                        out=rp, in0=vals, scalar=t8b[:, 7:8], in1=revi,
                        op0=ALU.is_ge, op1=ALU.mult)
                    rp2 = smp.tile([128, NCAND], F32, tag="scr")
                    w16 = smp.tile([128, 16], F32, tag="w16")
                    nc.vector.max(w16[:, 0:8], rp)
                    nc.vector.match_replace(rp2, w16[:, 0:8], rp, 0.0)
                    nc.vector.max(w16[:, 8:16], rp2)
                    cjf = smp.tile([128, 16], F32, tag="cjf")
                    nc.vector.tensor_scalar(
                        out=cjf, in0=w16, scalar1=-1.0, scalar2=float(N),
                        op0=ALU.mult, op1=ALU.add)
                    # idx wrap table: DRAM round-trip shuffle (i%16 -> stripe),
                    # PE stripe-broadcast, DVE col shuffle + int16 convert
                    idxd = iddp.tile([2048], F32)
                    nc.sync.dma_start(
                        out=idxd.rearrange("(ch g q) -> g ch q", ch=16, g=8, q=16),
                        in_=cjf)
                    M = ixp.tile([16, 128], F32, tag="M")
                    nc.sync.dma_start(
                        out=M, in_=idxd.rearrange("(ch c) -> ch c", ch=16))
                    Pb = pibp.tile([128, 128], F32, tag="Pb")
                    nc.tensor.matmul(Pb, lhsT=sb["s16"], rhs=M, start=True,
                                     stop=True)
                    idxs = ixp.tile([128, 128], I16, tag="idxs")
                    nc.scalar.activation(
                        out=idxs.rearrange("p (h q g) -> p h q g", h=2, q=8, g=8),
                        in_=Pb.rearrange("p (g h q) -> p h q g", g=8, h=2, q=8),
                        func=AF.Copy)
                    # gather all 2048 edge v-rows as two 1024-idx batches
                    # (bf16 dup-rows: 256B elements, halves nothing but keeps
                    # vg in bf16 for the 4x pre1 STT)
                    vg = vgp.tile([128, K * 2 * C], BF16, tag="vg")
                    vgv = vg.rearrange("p (k dd) -> p k dd", dd=2 * C)
                    for hh in range(2):
                        nc.gpsimd.dma_gather(
                            out_ap=vgv[:, 8 * hh:8 * (hh + 1), :],
                            in_ap=v_dram,
                            idxs_ap=idxs[:, 64 * hh:64 * (hh + 1)],
                            num_idxs=1024,
                            num_idxs_reg=nidx_reg,
                            elem_size=2 * C,
                            queue_num=t % 4,
                        )
                    pend[t] = (vg, vgv)

                def stage2(t):
                    i0 = 128 * t
                    vg, vgv = pend.pop(t)
                    # pre-activation: vg + u_i (broadcast over k), all-bf16
                    # packed operands -> 2x DVE perf mode (TT only; STT with a
                    # tensor in1 gets no fast modes); GELU -> bf16
                    pre1 = edp.tile([128, K * D], BF16, tag="pre1")
                    ub = u_r[:, D * t:D * (t + 1)].unsqueeze(1).broadcast_to(
                        [128, K, D])
                    nc.vector.tensor_tensor(
                        out=pre1.rearrange("p (k d) -> p k d", d=D),
                        in0=vgv[:, :, 0:D], in1=ub, op=ALU.add)
                    h1 = edp.tile([128, K * D], BF16, tag="h1")
                    nc.scalar.activation(h1, pre1, AF.Gelu)
                    # transpose k-pair blocks (bf16 PE transpose, bf16 PSUM);
                    # four transposes share a PSUM tile -> one ACT copy each
                    h1T2 = edp.tile([128, 8 * 128], BF16, tag="h1T2")
                    for j4 in range(2):
                        tp4 = ptrp.tile([128, 512], BF16, tag="tr")
                        for jj in range(4):
                            j = 4 * j4 + jj
                            nc.tensor.transpose(
                                tp4[:, 128 * jj:128 * (jj + 1)],
                                h1[:, 128 * j:128 * (j + 1)], sb["idb"])
                        nc.scalar.activation(h1T2[:, 512 * j4:512 * (j4 + 1)], tp4,
                                             AF.Copy)
                    # layer-2: 4 bf16 matmuls, k-parity on partition halves;
                    # single [128, 1024] gelu2 (ACT cost is free-size cycles:
                    # 64-partition ops waste half the engine)
                    p2 = p2p.tile([128, 1024], F32, tag="p2")
                    for s in range(2):
                        cs = slice(512 * s, 512 * (s + 1))
                        nc.tensor.matmul(
                            p2[0:64, cs], lhsT=sb["W2db"][0:64, :],
                            rhs=h1T2[0:64, cs], start=True, stop=True)
                        nc.tensor.matmul(
                            p2[64:128, cs], lhsT=sb["W2db"][64:128, :],
                            rhs=h1T2[64:128, cs], start=True, stop=True)
                    h2f = edp.tile([128, 1024], BF16, tag="h2f")
                    nc.scalar.activation(h2f, p2, AF.Gelu, bias=sb["b2d"])
                    # max over K: transpose FIRST (8 [128,128] bf16 PE
                    # transposes into one PSUM bank), then a pure free-dim
                    # fold tree [128,1024]->[128,64] at 2x bf16 -- also kills
                    # the old transpose-back step.  Layout after transpose:
                    # [i, (jblk, kpar, d)]; folds reduce jblk then kpar.
                    trh2 = ptr2.tile([128, 1024], BF16, tag="trh2")
                    for b2i in range(8):
                        nc.tensor.transpose(
                            trh2[:, 128 * b2i:128 * (b2i + 1)],
                            h2f[:, 128 * b2i:128 * (b2i + 1)], sb["idb"])
                    # fold1 may read only ONE operand from PSUM: stage the
                    # first half in SBUF (overlaps the remaining transposes)
                    s0 = edp.tile([128, 512], BF16, tag="s0")
                    nc.scalar.activation(s0, trh2[:, 0:512], AF.Copy)
                    s1 = edp.tile([128, 512], BF16, tag="s1")
                    nc.vector.tensor_tensor(
                        out=s1, in0=s0, in1=trh2[:, 512:1024], op=ALU.max)
                    s2 = edp.tile([128, 256], BF16, tag="s2")
                    nc.vector.tensor_tensor(
                        out=s2, in0=s1[:, 0:256], in1=s1[:, 256:512], op=ALU.max)
                    s3 = edp.tile([128, 128], BF16, tag="s3")
                    nc.vector.tensor_tensor(
                        out=s3, in0=s2[:, 0:128], in1=s2[:, 128:256], op=ALU.max)
                    orow = orp.tile([128, D], F32, tag="orow")
                    nc.vector.tensor_tensor(
                        out=orow, in0=s3[:, 0:64], in1=s3[:, 64:128], op=ALU.max)
                    nc.sync.dma_start(out=y[i0:i0 + 128, :], in_=orow)

                # software pipeline, 2 deep: the idx-shuffle/gather latency of
                # tile t hides behind the MLP work of tiles t-2/t-1
                LAG = 2
                for t in range(NT):
                    stage1(t)
                    if t >= LAG:
                        stage2(t - LAG)
                for t in range(NT - LAG, NT):
                    stage2(t)
    mybir.codegen_inst_isa_subclasses(nc)
    _split_excess_waits(nc)
    return nc


_NC = None


def kernel(features, W1, b1, W2, b2):
    global _NC
    features = np.ascontiguousarray(np.asarray(features, np.float32))
    consts = host_constants(W1, b1, W2, b2)
    if _NC is None:
        _NC = build_nc()
    in_maps = [{"x": features[c], **consts} for c in range(B)]
    res = run_bass_kernel_spmd(_NC, in_maps, core_ids=list(range(B)))
    return np.stack([res.results[c]["y"] for c in range(B)], axis=0)


if __name__ == "__main__":
    rng = np.random.default_rng(0)
    feats = rng.standard_normal((B, N, C)).astype(np.float32)
    W1 = (rng.standard_normal((2 * C, D)) * 0.05).astype(np.float32)
    b1 = np.zeros(D, np.float32)
    W2 = (rng.standard_normal((D, D)) * 0.05).astype(np.float32)
    b2 = np.zeros(D, np.float32)
    out = kernel(features=feats, W1=W1, b1=b1, W2=W2, b2=b2)
    print(out.shape, out.dtype)

